# revision 24
# baseline (speedup 1.0000x reference)
"""YOLOv2-style detection loss on 8 Trainium2 NeuronCores.

Contract: kernel(**inputs) takes the FULL unsharded inputs
(detection_result [2048,125,13,13] f32, gt_grid [2048,125,13,13] f32,
anchors_w [5] f32, anchors_h [5] f32) and returns the full output —
the tuple (loss, obj_loss, no_obj_loss, confidence) of f32 scalars,
matching the reference.

Strategy: pure data parallel over the batch axis; each core gets 256
batches. The host re-lays each core's shard channel-major so partition p
holds [125 channels x (2 batches x 169 cells)] — every channel slice is
a contiguous [128, 338] 2D access pattern on SBUF. The IoU/confidence
chain is computed k-batched ([128, 5, 338] with broadcast APs), class
and coord squared-error sums are reduced with fused
scalar_tensor_tensor accumulations against the object mask, and the
final cross-partition + cross-core reduction plus the masked-mean/λ
arithmetic happen on the host in float64.
"""

import numpy as np

B, CH, G = 2048, 125, 13
NB = 5
GG = G * G               # 169
NCORES = 8
BPC = B // NCORES        # 256 batches per core
P = 128                  # SBUF partitions
BPP = BPC // P           # 2 batches per partition row
W = BPP * GG             # 338: per-channel window per partition
ROW = CH * W             # 42250 f32 per partition row
LAMBDA_COORD, LAMBDA_NOOBJ = 5.0, 0.5

_CACHE = {}


def _build(aw, ah, cfg=None):
    """Build the single-core Bass program (anchors baked as immediates)."""
    from contextlib import ExitStack

    cfg = dict(cfg or {})
    kcls_bufs = cfg.get("kcls_bufs", 2)
    kco_bufs = cfg.get("kco_bufs", 2)
    big_bufs = cfg.get("big_bufs", 2)
    med_bufs = cfg.get("med_bufs", 2)
    act_small = cfg.get("act_small", True)    # +1/relu ops on ACT
    # how many of the 10 class-chunk subtracts run on DVE (rest on POOL)
    cls_sub_dve = cfg.get("cls_sub_dve", 0)
    coord_sub_eng = cfg.get("coord_sub_eng", "gpsimd")
    repeat = cfg.get("repeat", 1)

    import concourse.bacc as bacc
    import concourse.mybir as mybir
    import concourse.tile as tile

    f32 = mybir.dt.float32
    op = mybir.AluOpType
    AF = mybir.ActivationFunctionType

    nc = bacc.Bacc()
    det_t = nc.dram_tensor("det", [P, ROW], f32, kind="ExternalInput")
    gt_t = nc.dram_tensor("gt", [P, ROW], f32, kind="ExternalInput")
    # partial sums per partition: [S4tot, S4o, pad, pad, pad + Nk(5)] etc.
    outS = nc.dram_tensor("partsS", [P, 2], f32, kind="ExternalOutput")   # S4tot,S4o
    outN = nc.dram_tensor("partsN", [P, NB], f32, kind="ExternalOutput")  # Nk
    outC = nc.dram_tensor("partsC", [P, NB], f32, kind="ExternalOutput")  # Sc per k
    outT = nc.dram_tensor("partsT", [P, 2 * NB], f32, kind="ExternalOutput")  # Tk per (k,h)

    detv = det_t[:].rearrange("p (c w) -> p c w", w=W)   # [128, 125, 338]
    gtv = gt_t[:].rearrange("p (c w) -> p c w", w=W)

    with tile.TileContext(nc) as tc, ExitStack() as ctx:
        pers = ctx.enter_context(tc.tile_pool(name="pers", bufs=1))
        kco = ctx.enter_context(tc.tile_pool(name="kco", bufs=kco_bufs))
        kcls = ctx.enter_context(tc.tile_pool(name="kcls", bufs=kcls_bufs))
        big = ctx.enter_context(tc.tile_pool(name="big", bufs=big_bufs))
        med = ctx.enter_context(tc.tile_pool(name="med", bufs=med_bufs))
        chain = ctx.enter_context(tc.tile_pool(name="chain", bufs=1))
        colp = ctx.enter_context(tc.tile_pool(name="cols", bufs=1))

        def _add1(ap):
            if act_small:
                nc.scalar.add(ap, ap, 1.0)
            else:
                nc.gpsimd.tensor_scalar_add(ap, ap, 1.0)

        def _relu(ap):
            if act_small:
                nc.scalar.activation(ap, ap, AF.Relu)
            else:
                nc.gpsimd.tensor_scalar_max(ap, ap, 0.0)

        for _rep in range(repeat):
            # ---- persistent loads --------------------------------------
            # channel 25k+4 for all k, det and gt: [128, 5, 338]
            det4s = pers.tile([P, NB, W], f32, tag="det4s")
            gt4s = pers.tile([P, NB, W], f32, tag="gt4s")
            nc.sync.dma_start(det4s[:], detv[:, 4::25, :])
            nc.sync.dma_start(gt4s[:], gtv[:, 4::25, :])
            # anchor scalar vectors along the k free-dim (inline consts)
            awt = nc.inline_tensor(
                np.broadcast_to(np.asarray(aw, np.float32)[None, :], (1, NB)).copy(),
                name=f"awt{_rep}")
            aht = nc.inline_tensor(
                np.broadcast_to(np.asarray(ah, np.float32)[None, :], (1, NB)).copy(),
                name=f"aht{_rep}")
            avec = pers.tile([P, 2, NB], f32, tag="avec")
            nc.sync.dma_start(avec[:, 0], awt[:].broadcast_to([P, NB]))
            nc.sync.dma_start(avec[:, 1], aht[:].broadcast_to([P, NB]))

            # k=0 coord channels double as the global box-0 slices
            co_det0 = pers.tile([P, 4, W], f32, tag="co_det0")
            co_gt0 = pers.tile([P, 4, W], f32, tag="co_gt0")
            nc.sync.dma_start(co_det0[:], detv[:, 0:4, :])
            nc.sync.dma_start(co_gt0[:], gtv[:, 0:4, :])
            px, py = co_det0[:, 0], co_det0[:, 1]
            c2, c3 = co_det0[:, 2], co_det0[:, 3]
            gx, gy = co_gt0[:, 0], co_gt0[:, 1]
            gw, gh = co_gt0[:, 2], co_gt0[:, 3]

            # ---- anchor-independent pieces ([128, 338]) ----------------
            x1 = pers.tile([P, W], f32, tag="x1")
            y1 = pers.tile([P, W], f32, tag="y1")
            a2 = pers.tile([P, W], f32, tag="a2")
            v1 = chain.tile([P, W], f32, tag="v1")
            nc.vector.tensor_max(x1[:], px, gx)
            nc.vector.tensor_max(y1[:], py, gy)
            nc.vector.tensor_sub(v1[:], gw, gx)
            _add1(v1[:])
            nc.vector.tensor_sub(a2[:], gh, gy)
            _add1(a2[:])
            nc.vector.tensor_mul(a2[:], v1[:], a2[:])

            colsS = colp.tile([P, 2], f32, tag="colsS")
            colsN = colp.tile([P, NB], f32, tag="colsN")
            colsC = colp.tile([P, NB], f32, tag="colsC")
            colsT = colp.tile([P, 2 * NB], f32, tag="colsT")

            def b_k(ap2d):        # [128,338] -> broadcast [128,5,338]
                return ap2d.unsqueeze(1).broadcast_to([P, NB, W])

            # ---- k-batched IoU/conf chain ([128, 5, 338]) --------------
            pw = chain.tile([P, NB, W], f32, tag="pw")
            ph = chain.tile([P, NB, W], f32, tag="ph")
            ta = chain.tile([P, NB, W], f32, tag="ta")
            tb = chain.tile([P, NB, W], f32, tag="tb")
            inter = chain.tile([P, NB, W], f32, tag="inter")
            a1 = chain.tile([P, NB, W], f32, tag="a1")
            aw_b = avec[:, 0].unsqueeze(2).broadcast_to([P, NB, W])
            ah_b = avec[:, 1].unsqueeze(2).broadcast_to([P, NB, W])
            nc.vector.tensor_mul(pw[:], b_k(c2), aw_b)
            nc.vector.tensor_mul(ph[:], b_k(c3), ah_b)
            # inter = ((x2-x1)+1)*((y2-y1)+1)
            nc.vector.tensor_tensor(ta[:], pw[:], b_k(gw), op=op.min)
            nc.vector.tensor_sub(ta[:], ta[:], b_k(x1[:]))
            _add1(ta[:])
            nc.vector.tensor_tensor(tb[:], ph[:], b_k(gh), op=op.min)
            nc.vector.tensor_sub(tb[:], tb[:], b_k(y1[:]))
            _add1(tb[:])
            nc.vector.tensor_mul(inter[:], ta[:], tb[:])
            # a1 = ((pw-px)+1)*((ph-py)+1)
            nc.vector.tensor_sub(ta[:], pw[:], b_k(px))
            _add1(ta[:])
            nc.vector.tensor_sub(tb[:], ph[:], b_k(py))
            _add1(tb[:])
            nc.vector.tensor_mul(a1[:], ta[:], tb[:])
            # den = (a1 + a2) - inter; iou = inter * (1/den)
            nc.vector.tensor_add(a1[:], a1[:], b_k(a2[:]))
            nc.vector.tensor_sub(a1[:], a1[:], inter[:])
            nc.vector.reciprocal(a1[:], a1[:])
            nc.vector.tensor_mul(inter[:], inter[:], a1[:])
            _relu(inter[:])
            # conf = det4 * relu(iou); d4 = conf - gt4; sq4 (+ total sum)
            nc.vector.tensor_mul(inter[:], det4s[:], inter[:])
            nc.vector.tensor_sub(inter[:], inter[:], gt4s[:])
            nc.scalar.activation(inter[:], inter[:], AF.Square,
                                 accum_out=colsS[:, 0:1])
            nc.vector.scalar_tensor_tensor(
                a1[:], inter[:], 1.0, gt4s[:], op.mult, op.mult,
                accum_out=colsS[:, 1:2])
            # Nk per anchor: one grouped reduce over the window axis
            nc.vector.reduce_sum(colsN[:], gt4s[:], axis=mybir.AxisListType.X)

            # ---- per-anchor coord + class masked sums ------------------
            ndve = 0
            for k in range(NB):
                obj = gt4s[:, k]                       # [128, 338]
                # coords: channels 25k .. 25k+3
                if k == 0:
                    dsrc, gsrc = co_det0[:], co_gt0[:]
                else:
                    dc_d = kco.tile([P, 4, W], f32, tag="dc_d")
                    dc_g = kco.tile([P, 4, W], f32, tag="dc_g")
                    nc.sync.dma_start(dc_d[:], detv[:, 25 * k:25 * k + 4, :])
                    nc.sync.dma_start(dc_g[:], gtv[:, 25 * k:25 * k + 4, :])
                    dsrc, gsrc = dc_d[:], dc_g[:]
                dc = med.tile([P, 4, W], f32, tag="dc")
                getattr(nc, coord_sub_eng).tensor_sub(dc[:], dsrc, gsrc)
                nc.scalar.square(dc[:], dc[:])
                nc.vector.scalar_tensor_tensor(
                    dc[:], dc[:], 1.0,
                    obj.unsqueeze(1).broadcast_to([P, 4, W]),
                    op.mult, op.mult, accum_out=colsC[:, k:k + 1])

                # classes: channels 25k+5 .. 25k+24 in two 10-channel chunks
                for h in range(2):
                    c0 = 25 * k + 5 + 10 * h
                    cl_d = kcls.tile([P, 10, W], f32, tag="cl_d")
                    cl_g = kcls.tile([P, 10, W], f32, tag="cl_g")
                    nc.sync.dma_start(cl_d[:], detv[:, c0:c0 + 10, :])
                    nc.sync.dma_start(cl_g[:], gtv[:, c0:c0 + 10, :])
                    dcl = big.tile([P, 10, W], f32, tag="dcl")
                    if ndve < cls_sub_dve:
                        nc.vector.tensor_sub(dcl[:], cl_d[:], cl_g[:])
                    else:
                        nc.gpsimd.tensor_sub(dcl[:], cl_d[:], cl_g[:])
                    ndve += 1
                    nc.scalar.square(dcl[:], dcl[:])
                    nc.vector.scalar_tensor_tensor(
                        dcl[:], dcl[:], 1.0,
                        obj.unsqueeze(1).broadcast_to([P, 10, W]),
                        op.mult, op.mult,
                        accum_out=colsT[:, 2 * k + h:2 * k + h + 1])

            nc.sync.dma_start(outS[:], colsS[:])
            nc.sync.dma_start(outN[:], colsN[:])
            nc.sync.dma_start(outC[:], colsC[:])
            nc.sync.dma_start(outT[:], colsT[:])
    nc.finalize()
    return nc


BEST_CFG = {}


def _get_program(aw, ah):
    key = (tuple(np.asarray(aw, np.float32).tolist()),
           tuple(np.asarray(ah, np.float32).tolist()))
    if key not in _CACHE:
        _CACHE[key] = _build(np.asarray(aw, np.float32),
                             np.asarray(ah, np.float32), BEST_CFG)
    return _CACHE[key]


def _shard(arr):
    """[2048, 125, 169] -> per-core channel-major [128, ROW] shards."""
    out = []
    for c in range(NCORES):
        s = arr[c * BPC:(c + 1) * BPC].reshape(P, BPP, CH, GG)
        s = np.ascontiguousarray(s.transpose(0, 2, 1, 3))  # [128, 125, 2, 169]
        out.append(s.reshape(P, ROW))
    return out


def kernel(detection_result, gt_grid, anchors_w, anchors_h):
    from concourse.bass_utils import run_bass_kernel_spmd

    det = np.ascontiguousarray(np.asarray(detection_result, np.float32)).reshape(B, CH, GG)
    gt = np.ascontiguousarray(np.asarray(gt_grid, np.float32)).reshape(B, CH, GG)

    nc = _get_program(anchors_w, anchors_h)

    det_sh = _shard(det)
    gt_sh = _shard(gt)
    in_maps = [{"det": det_sh[c], "gt": gt_sh[c]} for c in range(NCORES)]

    res = run_bass_kernel_spmd(nc, in_maps, core_ids=list(range(NCORES)))

    S4tot = 0.0
    S4o = 0.0
    Nk = np.zeros(NB, np.float64)
    Sc = np.zeros(NB, np.float64)
    Tk = np.zeros(NB, np.float64)
    for r in res.results:
        s = r["partsS"].astype(np.float64).sum(axis=0)
        S4tot += s[0]
        S4o += s[1]
        Nk += r["partsN"].astype(np.float64).sum(axis=0)
        Sc += r["partsC"].astype(np.float64).sum(axis=0)
        Tk += r["partsT"].astype(np.float64).sum(axis=0).reshape(NB, 2).sum(axis=1)

    N = Nk.sum()
    Ctot = float(B * NB * GG)
    Snoobj = S4tot - S4o
    coord = Sc.sum() / N if N > 0 else 0.0
    obj_loss = LAMBDA_COORD * coord + (S4o / N if N > 0 else 0.0)
    cn = Ctot - N
    no_obj_loss = LAMBDA_NOOBJ * (Snoobj / cn if cn > 0 else 0.0)
    confidence = float(sum((Tk[k] / Nk[k]) if Nk[k] > 0 else 0.0 for k in range(NB)))
    loss = obj_loss + no_obj_loss + confidence
    return (np.float32(loss), np.float32(obj_loss),
            np.float32(no_obj_loss), np.float32(confidence))
